# revision 2
# baseline (speedup 1.0000x reference)
"""Trainium2 Bass kernel for nn_AttentionSpikingNetwork (B=64, S=512).

Data-parallel over batch across 8 NeuronCores (8 batch elems per core).

Precision plan (validated against an exact numpy FP22/fp8 simulation of the
kernel numerics; harness budget 2e-2, this config sims at 1.27e-2 with zero
spk3 flips and the sim matched hardware to 7 digits on the previous config):
  - embed: hi pass in fp32r (wEh_m11 @ x_m11) + BOTH lo compensation terms
    (wEh@xl + wEl@xh) in a single fp8e4m3 DoubleRow psum group at 0.5
    cyc/row, sharing a 2^13 operand scale; combined at the drain as
    hi + 2^-13*lo. DoubleRow operands are host-packed [128, 2, *] tiles
    (contract dim padded 784->1024).
  - V, attention, cur3: single fp32r hi pass (study: 6.4e-3 combined).
  - cur2: full 3-pass fp32r (dropping it costs 1.6e-2 - too risky).
  - Q/K: single-pass packed weight (Wq|Wk in one 128-col block, one
    5-matmul chain); scores single-pass FP22 (softmax cancels the
    common-mode rounding).
Other structure:
  - Activations flow transposed ([feat, seq]); scores transposed (K @ Q.T);
    softmax without max-subtraction; exp written as F32R by the ACT engine.
  - softmax denominator folded into the attention matmul: vh carries a
    601st all-ones column, so attention chunk 4 (emitted first) yields
    den in psum row 88; reciprocal_approx_fast + gpsimd broadcast run
    under attention chunks 0-3.
  - bv folded into b2 host-side (b2' = b2 + W2 @ bv).
  - elem b+1's embed j-chunks are emitted between b's scores and attention
    as PE filler for the exp/normalize chains.
"""
import os
import sys

for _p in ("/opt/trn_rl_repo", "/root/.axon_site/_ro/trn_rl_repo"):
    if os.path.isdir(_p) and _p not in sys.path:
        sys.path.insert(0, _p)

import numpy as np
from contextlib import ExitStack

import concourse.bass as bass
import concourse.bass_isa as bass_isa
import concourse.bacc as bacc
import concourse.mybir as mybir
import concourse.tile as tile
from concourse.bass_utils import run_bass_kernel_spmd

F32 = mybir.dt.float32
F32R = mybir.dt.float32r
F8 = mybir.dt.float8e4
AF = mybir.ActivationFunctionType
OP = mybir.AluOpType
DR = mybir.MatmulPerfMode.DoubleRow

NCORES = 8
B, S, DIN, DEMB, DQK, DH2, DOUT = 64, 512, 784, 600, 64, 200, 10
NB = B // NCORES   # batch elems per core
DINP = 896         # DIN zero-padded so every hi chunk contracts 128 (the
                   # 16-wide tail chunk clocks at 460ns vs 277ns)
DIN8 = 1024        # DIN padded for fp8 DoubleRow chunks of 256
NKK = DIN8 // 256  # 4 DoubleRow contract chunks
DEBUG_S1 = False
DEBUG_MID = False
SC8 = np.float32(8192.0)   # 2^13 lo-operand scale
ISC8 = float(1.0 / 8192.0)


def _chunks(total, step=128):
    return [(i, min(step, total - i)) for i in range(0, total, step)]

CH_DIN = _chunks(DINP)   # 7 chunks of 128 (zero-padded from 784)
CH_EMB = _chunks(DEMB)   # 5
CH_H2 = _chunks(DH2)     # 2
CH_S = _chunks(S)        # 4
CH_VN = [(0, 344), (344, 256)]  # V free-dim split; both >=256 keeps fp32r full-rate


def _pad32(n):
    """DoubleRow LDWEIGHTS needs 32-aligned column counts (88 -> 96)."""
    return (n + 31) // 32 * 32


def round_m11(a):
    """Round fp32 to 11 explicit mantissa bits (fp32r/FP22 grid), RNE."""
    a = np.ascontiguousarray(a, np.float32)
    u = a.view(np.uint32).astype(np.uint64)
    r = (u + 0x7FF + ((u >> 12) & 1)) & np.uint64(0xFFFFF000)
    return r.astype(np.uint32).view(np.float32)


def _split(a):
    hi = round_m11(a)
    lo = (a.astype(np.float32) - hi).astype(np.float32)
    return hi, lo


def build_nc(nb=NB):
    nc = bacc.Bacc()

    def par(name, shape, dt=F32R, out=False):
        return nc.declare_dram_parameter(name, list(shape), dt, isOutput=out)

    xh = par("xh", [nb, DINP, S])
    x8h = par("x8h", [nb, NKK, 128, 2, S], F8)   # e4m3(xh), DR-packed
    x8l = par("x8l", [nb, NKK, 128, 2, S], F8)   # e4m3(xl*2^13), DR-packed
    wEh = par("wEh", [DINP * DEMB])
    demb8 = sum(_pad32(cn) for _, cn in CH_EMB)  # 608: col-padded chunks
    wE8h = par("wE8h", [NKK * 256 * demb8], F8)  # e4m3(wEh), DR blocks
    wE8l = par("wE8l", [NKK * 256 * demb8], F8)  # e4m3(wEl*2^13), DR blocks
    wQK = par("wQK", [DEMB, 128])
    wVh = par("wVh", [DEMB, DEMB])
    w2h = par("w2h", [DEMB * DH2])
    w2l = par("w2l", [DEMB * DH2])
    w3h = par("w3h", [DH2, DOUT])
    bE = par("bE", [DEMB, 1], F32)
    bQK = par("bQK", [128, 1], F32)
    b2 = par("b2", [DH2, 1], F32)
    b3 = par("b3", [DOUT, 1], F32)
    if DEBUG_S1:
        os1 = par("os1", [nb, DEMB, S], F32, out=True)
    if DEBUG_MID:
        oqh = par("oqh", [128, S], F32, out=True)
        okh = par("okh", [128, S], F32, out=True)
        opth = par("opth", [128, S], F32, out=True)
        oinv = par("oinv", [128, S], F32, out=True)
        os2h = par("os2h", [DEMB, S], F32, out=True)
        ospk2 = par("ospk2", [DH2, S], F32, out=True)
    os_ = par("os", [nb, DOUT, S], F32, out=True)
    om_ = par("om", [nb, DOUT, S], F32, out=True)

    with ExitStack() as ctx:
        tc = ctx.enter_context(tile.TileContext(nc))
        wp = ctx.enter_context(tc.tile_pool(name="wp", bufs=1))
        xp = ctx.enter_context(tc.tile_pool(name="xp", bufs=2))
        sp = ctx.enter_context(tc.tile_pool(name="sp", bufs=1))
        outp = ctx.enter_context(tc.tile_pool(name="outp", bufs=1))
        ps_em = ctx.enter_context(tc.tile_pool(name="ps_em", bufs=1, space="PSUM"))
        ps = ctx.enter_context(tc.tile_pool(name="ps", bufs=5, space="PSUM"))

        # ---- resident weights / consts ----
        # Weight DMAs are emitted j-major so batch elem 0's first embed
        # j-chunk has its blocks within ~1MB of DMA; the rest stream in
        # during elem 0's embed compute.
        wEh_t = {}     # (k, j) -> [kn, cn] f32r block
        wE8h_t = {}    # (kk, j) -> [128, 2, cn] fp8 DR block
        wE8l_t = {}

        def _emit_embed_weight_dmas():
            offs_h = {}
            off = 0
            for k, (k0, kn) in enumerate(CH_DIN):
                for j, (c0, cn) in enumerate(CH_EMB):
                    offs_h[(k, j)] = (off, kn, cn)
                    off += kn * cn
            offs_8 = {}
            off = 0
            for kk in range(NKK):
                for j, (c0, cn) in enumerate(CH_EMB):
                    offs_8[(kk, j)] = (off, _pad32(cn))
                    off += 256 * _pad32(cn)
            for j, (c0, cn) in enumerate(CH_EMB):
                for k, (k0, kn) in enumerate(CH_DIN):
                    t = wp.tile([kn, cn], F32R, name=f"wEh_{k}_{j}",
                                tag=f"wEh_{k}_{j}")
                    o, _, _ = offs_h[(k, j)]
                    nc.scalar.dma_start(out=t, in_=wEh[o:o + kn * cn].rearrange(
                        "(a b) -> a b", b=cn))
                    wEh_t[(k, j)] = t
                for kk in range(NKK):
                    for nm, dram, store in (("wE8h", wE8h, wE8h_t),
                                            ("wE8l", wE8l, wE8l_t)):
                        o, cp = offs_8[(kk, j)]
                        t = wp.tile([128, 2, cp], F8, name=f"{nm}_{kk}_{j}",
                                    tag=f"{nm}_{kk}_{j}")
                        nc.scalar.dma_start(
                            out=t, in_=dram[o:o + 256 * cp].rearrange(
                                "(p a b) -> p a b", a=2, b=cp))
                        store[(kk, j)] = t

        def wtiles(dram, chs, width, nm):
            hs = []
            for i, (c0, cn) in enumerate(chs):
                t = wp.tile([cn, width], F32R, name=f"{nm}{i}", tag=f"{nm}{i}")
                nc.scalar.dma_start(out=t, in_=dram[c0:c0 + cn, :])
                hs.append(t)
            return hs

        def wtiles2(dram, rchs, cchs, nm):
            out = {}
            off = 0
            for i, (r0, rn) in enumerate(rchs):
                for j, (c0, cn) in enumerate(cchs):
                    t = wp.tile([rn, cn], F32R, name=f"{nm}_{i}_{j}",
                                tag=f"{nm}_{i}_{j}")
                    nc.scalar.dma_start(
                        out=t, in_=dram[off:off + rn * cn].rearrange(
                            "(a b) -> a b", b=cn))
                    out[(i, j)] = t
                    off += rn * cn
            return out

        def btiles(dram, chs, nm):
            hs = []
            for i, (c0, cn) in enumerate(chs):
                t = wp.tile([cn, 1], F32, name=f"{nm}{i}", tag=f"{nm}{i}")
                nc.scalar.dma_start(out=t, in_=dram[c0:c0 + cn, :])
                hs.append(t)
            return hs

        _rest = {}

        def _load_rest():
            _rest["wQK"] = wtiles(wQK, CH_EMB, 128, "wQK")
            _rest["bQK"] = btiles(bQK, [(0, 128)], "bQK")[0]
            _rest["wVh"] = wtiles(wVh, CH_EMB, DEMB, "wVh")
            _rest["w2h"] = wtiles2(w2h, CH_EMB, CH_H2, "w2h")
            _rest["w2l"] = wtiles2(w2l, CH_EMB, CH_H2, "w2l")
            _rest["b2"] = btiles(b2, CH_H2, "b2")
            _rest["w3h"] = wtiles(w3h, CH_H2, DOUT, "w3h")
            _rest["b3"] = btiles(b3, [(0, DOUT)], "b3")[0]

        bE_t = btiles(bE, CH_EMB, "bE")

        MM = nc.tensor.matmul

        st = [dict() for _ in range(nb)]

        # ---- embed: j-major; per j-chunk one fp32r hi psum + one fp8-DR
        # lo psum, combined at the drain ----
        def emit_embed_x(b):
            xh_t = []
            for k, (k0, kn) in enumerate(CH_DIN):
                t = xp.tile([kn, S], F32R, name=f"xh{k}", tag=f"xh{k}")
                nc.sync.dma_start(out=t, in_=xh[b, k0:k0 + kn, :])
                xh_t.append(t)
            x8h_t, x8l_t = [], []
            for kk in range(NKK):
                th = xp.tile([128, 2, S], F8, name=f"x8h{kk}", tag=f"x8h{kk}")
                nc.sync.dma_start(out=th, in_=x8h[b, kk])
                x8h_t.append(th)
                tl = xp.tile([128, 2, S], F8, name=f"x8l{kk}", tag=f"x8l{kk}")
                nc.sync.dma_start(out=tl, in_=x8l[b, kk])
                x8l_t.append(tl)
            st[b]["x"] = (xh_t, x8h_t, x8l_t)
            st[b]["s1"] = [None] * len(CH_EMB)

        def emit_embed_j(b, j):
            xh_t, x8h_t, x8l_t = st[b]["x"]
            c0, cn = CH_EMB[j]
            hi_ps = ps_em.tile([cn, S], F32, name=f"emh{j % 2}",
                               tag=f"emh{j % 2}")
            lo_ps = ps_em.tile([_pad32(cn), S], F32, name="eml", tag="eml")
            nk = len(CH_DIN)
            for k in range(nk):
                MM(hi_ps, wEh_t[(k, j)], xh_t[k], start=(k == 0),
                   stop=(k == nk - 1))
            for kk in range(NKK):
                MM(lo_ps, wE8h_t[(kk, j)], x8l_t[kk], start=(kk == 0),
                   stop=False, perf_mode=DR)
                MM(lo_ps, wE8l_t[(kk, j)], x8h_t[kk], start=False,
                   stop=(kk == NKK - 1), perf_mode=DR)
            # s1 = (hi + 2^-13*lo + bE > 0.5), done as hi > thr with
            # thr = (0.5 - bE) - 2^-13*lo so each DVE op reads one psum
            # (bE2 = 0.5 - bE is precomputed host-side)
            thr = sp.tile([cn, S], F32, name="emthr", tag="emthr", bufs=2)
            nc.vector.tensor_scalar(thr, lo_ps[0:cn, :], -ISC8, bE_t[j],
                                    OP.mult, OP.add)
            t = sp.tile([cn, S], F32R, name=f"s1_{j}", tag=f"s1_{j}", bufs=2)
            nc.vector.tensor_tensor(t, hi_ps, thr, OP.is_gt)
            if DEBUG_S1:
                nc.sync.dma_start(out=os1[b, c0:c0 + cn, :],
                                  in_=t.bitcast(F32))
            st[b]["s1"][j] = t

        def emit_qk(b):
            s1_t = st[b]["s1"]
            wQK_t = _rest["wQK"]
            q_ps = ps.tile([128, S], F32, name="qk_ps", tag="ps")
            n = len(CH_EMB)
            for i in range(n):
                MM(q_ps, wQK_t[i], s1_t[i], start=(i == 0), stop=(i == n - 1))
            # Q rows 0:64, K rows 64:128 -> zero-padded 128-contraction tiles
            qh = sp.tile([128, S], F32R, name="qh", tag="qh")
            kh = sp.tile([128, S], F32R, name="kh", tag="kh")
            if b == 0:
                # zero rows 64:128 once (reused across batch elems); memset
                # can't write F32R, so multiply finite psum rows by 0
                nc.vector.tensor_scalar(qh[64:128, :], q_ps[64:128, :],
                                        0.0, None, OP.mult)
                nc.vector.tensor_scalar(kh[64:128, :], q_ps[64:128, :],
                                        0.0, None, OP.mult)
            nc.vector.tensor_scalar(qh[0:64, :], q_ps[0:64, :],
                                    _rest["bQK"][0:64, :], None, OP.add)
            nc.vector.tensor_scalar(kh[0:64, :], q_ps[64:128, :],
                                    _rest["bQK"][64:128, :], None, OP.add)
            if DEBUG_MID and b == 0:
                nc.sync.dma_start(out=oqh[:, :], in_=qh.bitcast(F32))
                nc.sync.dma_start(out=okh[:, :], in_=kh.bitcast(F32))
            st[b].update(kh=kh, qh=qh)

        def emit_V(b):
            s1_t = st[b]["s1"]
            wVh_t = _rest["wVh"]
            vh_t = []
            for ti, (t0, tn) in enumerate(CH_S):
                v_ps = [ps.tile([tn, w], F32, name=f"v_ps{j}", tag="ps")
                        for j, (v0, w) in enumerate(CH_VN)]
                n = len(CH_EMB)
                for i in range(n):
                    lh = s1_t[i][:, t0:t0 + tn]
                    for j, (v0, w) in enumerate(CH_VN):
                        MM(v_ps[j], lh, wVh_t[i][:, v0:v0 + w],
                           start=(i == 0), stop=(i == n - 1))
                # 609-wide: cols 600:608 zero, col 608 ones — attention
                # chunk 4 then yields the softmax denominator at psum
                # partition 96 (a DVE-alignable base; 88 is not)
                vh = sp.tile([tn, DEMB + 9], F32R, name=f"vh{ti}",
                             tag=f"vh{ti}")
                for j, (v0, w) in enumerate(CH_VN):
                    nc.scalar.activation(vh[:, v0:v0 + w], v_ps[j], AF.Copy)
                nc.vector.tensor_scalar(vh[:, DEMB:DEMB + 8],
                                        v_ps[0][:, 0:8], 0.0, None, OP.mult)
                nc.vector.tensor_scalar(vh[:, DEMB + 8:DEMB + 9],
                                        v_ps[0][:, 0:1], 0.0, 1.0,
                                        OP.mult, OP.add)
                vh_t.append(vh)
            st[b]["vh"] = vh_t

        def emit_scores(b):
            qh, kh = st[b]["qh"], st[b]["kh"]
            pth_t = []
            for ti, (t0, tn) in enumerate(CH_S):
                scT_ps = ps.tile([tn, S], F32, name=f"scT_ps{ti}", tag="ps")
                MM(scT_ps, kh[:, t0:t0 + tn], qh, start=True, stop=True)
                expT = sp.tile([tn, S], F32R, name=f"expT{ti}",
                               tag=f"expT{ti}", bufs=2)
                nc.scalar.activation(expT, scT_ps, AF.Exp, scale=0.125)
                pth_t.append(expT)
            if DEBUG_MID and b == 0:
                nc.sync.dma_start(out=opth[:, :], in_=pth_t[0].bitcast(F32))
            st[b].update(pth=pth_t)

        def emit_attn_num(b):
            """attention numerator chunk 4 first (89 rows incl. den), then
            chunks 0-2; the den reciprocal+broadcast runs under them."""
            vh_t = st[b]["vh"]
            pth_t = st[b]["pth"]
            nt = len(CH_S)
            ao = {}
            for i in (4, 0, 1, 2):
                c0, cn = CH_EMB[i]
                w = cn + 9 if i == 4 else cn   # 88 V rows + 8 zeros + den
                t = ps.tile([w, S], F32, name=f"ao_ps{i}", tag="ps")
                for ti in range(nt):
                    MM(t, vh_t[ti][:, c0:c0 + w], pth_t[ti],
                       start=(ti == 0), stop=(ti == nt - 1))
                ao[i] = t
                if i == 4:
                    # copy den to a base-0 SBUF tile first: the custom-DVE
                    # reciprocal does not honor the psum partition offset
                    dent = sp.tile([1, S], F32, name="dent", tag="dent",
                                   bufs=2)
                    nc.vector.tensor_copy(dent, t[96:97, :])
                    invs = sp.tile([1, S], F32, name="invs", tag="invs",
                                   bufs=2)
                    nc.vector.reciprocal_approx_fast(out=invs, in_=dent)
                    invb = sp.tile([128, S], F32, name="invb", tag="invb",
                                   bufs=2)
                    nc.gpsimd.partition_broadcast(invb, invs)
                    if DEBUG_MID and b == 0:
                        nc.sync.dma_start(out=oinv[:, :], in_=invb)
                    st[b]["invb"] = invb
            st[b]["ao"] = ao

        def _drain_s2(b, i):
            s1_t = st[b]["s1"]
            invb = st[b]["invb"]
            ao = st[b]["ao"]
            c0, cn = CH_EMB[i]
            raw = sp.tile([cn, S], F32, name="s2raw", tag="s2raw", bufs=2)
            nc.vector.scalar_tensor_tensor(raw, ao[i][0:cn, :], 0.0,
                                           invb[0:cn, :], OP.add, OP.mult)
            # s2 = raw + s1 (bv folded into b2); f32 then split h/l
            f = sp.tile([cn, S], F32, name="s2f", tag="s2f", bufs=2)
            nc.vector.tensor_tensor(f, raw, s1_t[i].bitcast(F32), OP.add)
            h = sp.tile([cn, S], F32R, name=f"s2h{i}", tag=f"s2h{i}")
            nc.scalar.activation(h, f, AF.Copy)
            l = sp.tile([cn, S], F32R, name=f"s2l{i}", tag=f"s2l{i}")
            nc.vector.tensor_tensor(l, f, h.bitcast(F32), OP.subtract)
            if DEBUG_MID and b == 0:
                c0_, cn_ = CH_EMB[i]
                nc.sync.dma_start(out=os2h[c0_:c0_ + cn_, :],
                                  in_=h.bitcast(F32))
            st[b]["s2h"][i] = h
            st[b]["s2l"][i] = l

        def emit_attn_tail(b):
            vh_t = st[b]["vh"]
            pth_t = st[b]["pth"]
            nt = len(CH_S)
            ao = st[b]["ao"]
            st[b]["s2h"] = [None] * 5
            st[b]["s2l"] = [None] * 5

            # with 5 psum buffers ao[3] gets its own bank; drains follow
            c0, cn = CH_EMB[3]
            t = ps.tile([cn, S], F32, name="ao_ps3", tag="ps")
            for ti in range(nt):
                MM(t, vh_t[ti][:, c0:c0 + cn], pth_t[ti],
                   start=(ti == 0), stop=(ti == nt - 1))
            ao[3] = t
            for i in (4, 0, 1, 2, 3):
                _drain_s2(b, i)
            s2h_t, s2l_t = st[b]["s2h"], st[b]["s2l"]

            # cur2.T = W2 @ spk2_in.T (3-pass fp32r) -> spk2
            s2_t = []
            for hi, (h0, hn) in enumerate(CH_H2):
                c2_ps = ps.tile([hn, S], F32, name=f"c2_ps{hi}", tag="ps")
                n = len(CH_EMB)
                for i in range(n):
                    wh = _rest["w2h"][(i, hi)]
                    wl = _rest["w2l"][(i, hi)]
                    MM(c2_ps, wh, s2h_t[i], start=(i == 0), stop=False)
                    MM(c2_ps, wh, s2l_t[i], start=False, stop=False)
                    MM(c2_ps, wl, s2h_t[i], start=False, stop=(i == n - 1))
                t2 = sp.tile([hn, S], F32R, name=f"spk2_{hi}", tag=f"spk2_{hi}")
                nc.vector.tensor_scalar(t2, c2_ps, _rest["b2"][hi], 0.3,
                                        OP.add, OP.is_gt)
                if DEBUG_MID and b == 0:
                    nc.sync.dma_start(out=ospk2[h0:h0 + hn, :],
                                      in_=t2.bitcast(F32))
                s2_t.append(t2)

            # cur3.T = W3 @ spk2.T (single pass) -> outputs
            c3_ps = ps.tile([DOUT, S], F32, name="c3_ps", tag="ps")
            n = len(CH_H2)
            for hi in range(n):
                MM(c3_ps, _rest["w3h"][hi], s2_t[hi], start=(hi == 0),
                   stop=(hi == n - 1))
            spk3_t = outp.tile([DOUT, S], F32, name="spk3_t", tag="spk3_t")
            c3b_t = outp.tile([DOUT, S], F32, name="c3b_t", tag="c3b_t")
            mem3_t = outp.tile([DOUT, S], F32, name="mem3_t", tag="mem3_t")
            nc.vector.tensor_scalar(spk3_t, c3_ps, _rest["b3"], 0.3,
                                    OP.add, OP.is_gt)
            nc.vector.tensor_scalar(c3b_t, c3_ps, _rest["b3"], None, OP.add)
            nc.vector.scalar_tensor_tensor(mem3_t, spk3_t, -0.3, c3b_t,
                                           OP.mult, OP.add)
            nc.sync.dma_start(out=os_[b, :, :], in_=spk3_t)
            nc.sync.dma_start(out=om_[b, :, :], in_=mem3_t)

        _emit_embed_weight_dmas()
        emit_embed_x(0)
        for j in range(len(CH_EMB)):
            emit_embed_j(0, j)
            if j == 0:
                _load_rest()
        for b in range(nb):
            emit_qk(b)
            if b == nb - 1:
                emit_scores(b)
            emit_V(b)
            if b + 1 < nb:
                emit_embed_x(b + 1)
                emit_embed_j(b + 1, 0)
                emit_scores(b)
                emit_embed_j(b + 1, 1)
                emit_attn_num(b)
                emit_embed_j(b + 1, 2)
                emit_embed_j(b + 1, 3)
                emit_embed_j(b + 1, 4)
            else:
                emit_attn_num(b)
            emit_attn_tail(b)

    nc.finalize()
    return nc


_NC_CACHE = {}


def _get_nc(nb):
    if nb not in _NC_CACHE:
        _NC_CACHE[nb] = build_nc(nb)
    return _NC_CACHE[nb]


def _to_f8(a):
    import ml_dtypes
    return np.asarray(a, np.float32).astype(ml_dtypes.float8_e4m3)


def _pack_dr_x(xT):
    """[DIN, S] -> [NKK, 128, 2, S], contract rows p + 128*j + 256*kk."""
    out = np.zeros((NKK, 128, 2, S), np.float32)
    src = np.zeros((DIN8, S), np.float32)
    src[:DIN] = xT
    for kk in range(NKK):
        blk = src[kk * 256:(kk + 1) * 256]         # [256, S]
        out[kk] = blk.reshape(2, 128, S).transpose(1, 0, 2)
    return out


def _pack_dr_w(w):
    """[DIN8, DEMB] -> flat [(kk, j) blocks of [128, 2, pad32(cn)]]."""
    parts = []
    for kk in range(NKK):
        blk = w[kk * 256:(kk + 1) * 256]           # [256, DEMB]
        blk = blk.reshape(2, 128, -1).transpose(1, 0, 2)  # [128, 2, DEMB]
        for (c0, cn) in CH_EMB:
            cp = _pad32(cn)
            sub = np.zeros((128, 2, cp), np.float32)
            sub[:, :, :cn] = blk[:, :, c0:c0 + cn]
            parts.append(sub.ravel())
    return np.concatenate(parts)


def make_in_maps(x, We, be, Wq, bq, Wk, bk, Wv, bv, W2, b2, W3, b3,
                 ncores=NCORES):
    x = np.ascontiguousarray(x, np.float32)
    if x.max() > 1.0:
        x = (x * np.float32(1.0 / 255.0)).astype(np.float32)

    def _pack_blocks(w, rchs, cchs):
        return np.concatenate(
            [w[r0:r0 + rn, c0:c0 + cn].ravel()
             for (r0, rn) in rchs for (c0, cn) in cchs])

    wEh_f, wEl_f = _split(np.ascontiguousarray(We.T))     # [DIN, DEMB]
    wEh_p = np.zeros((DINP, DEMB), np.float32)
    wEh_p[:DIN] = wEh_f
    wQKh, _ = _split(np.concatenate(
        [np.ascontiguousarray(Wq.T), np.ascontiguousarray(Wk.T)], axis=1))
    wVh_f, _ = _split(np.ascontiguousarray(Wv.T))
    w2h_f, w2l_f = _split(np.ascontiguousarray(W2.T))
    w3h_f, _ = _split(np.ascontiguousarray(W3.T))

    wE_pad_h = np.zeros((DIN8, DEMB), np.float32)
    wE_pad_h[:DIN] = wEh_f
    wE_pad_l = np.zeros((DIN8, DEMB), np.float32)
    wE_pad_l[:DIN] = wEl_f * SC8

    # bv folded into b2: cur2 = W2 @ (attn+spk1+bv) + b2 = W2@(attn+spk1)+b2'
    b2_eff = (b2.astype(np.float64)
              + W2.astype(np.float64) @ bv.astype(np.float64))

    shared = dict(
        wEh=_pack_blocks(wEh_p, CH_DIN, CH_EMB),
        wE8h=_to_f8(_pack_dr_w(wE_pad_h)),
        wE8l=_to_f8(_pack_dr_w(wE_pad_l)),
        wQK=wQKh, wVh=wVh_f,
        w2h=_pack_blocks(w2h_f, CH_EMB, CH_H2),
        w2l=_pack_blocks(w2l_f, CH_EMB, CH_H2),
        w3h=w3h_f,
        bE=np.ascontiguousarray((0.5 - be).reshape(-1, 1), np.float32),
        bQK=np.ascontiguousarray(
            np.concatenate([bq, bk]).reshape(-1, 1), np.float32),
        b2=np.ascontiguousarray(b2_eff.reshape(-1, 1), np.float32),
        b3=np.ascontiguousarray(b3.reshape(-1, 1), np.float32),
    )

    nb = x.shape[0] // ncores
    in_maps = []
    for c in range(ncores):
        xs = x[c * nb:(c + 1) * nb]                       # [nb, S, DIN]
        xT = np.ascontiguousarray(xs.transpose(0, 2, 1))  # [nb, DIN, S]
        xh_, xl_ = _split(xT)
        xh_p = np.zeros((nb, DINP, S), np.float32)
        xh_p[:, :DIN] = xh_
        x8h_ = np.stack([_pack_dr_x(xh_[e]) for e in range(nb)])
        x8l_ = np.stack([_pack_dr_x(xl_[e] * SC8) for e in range(nb)])
        in_maps.append(dict(shared, xh=xh_p,
                            x8h=_to_f8(x8h_), x8l=_to_f8(x8l_)))
    return in_maps, nb


def kernel(x, We, be, Wq, bq, Wk, bk, Wv, bv, W2, b2, W3, b3, _trace=False):
    args = [np.asarray(a, np.float32) for a in
            (x, We, be, Wq, bq, Wk, bk, Wv, bv, W2, b2, W3, b3)]
    in_maps, nb = make_in_maps(*args)
    nc = _get_nc(nb)
    res = run_bass_kernel_spmd(nc, in_maps, list(range(NCORES)), trace=_trace)
    spk3 = np.concatenate([r["os"].transpose(0, 2, 1) for r in res.results], 0)
    mem3 = np.concatenate([r["om"].transpose(0, 2, 1) for r in res.results], 0)
    kernel.last_results = res
    return (np.ascontiguousarray(spk3, np.float32),
            np.ascontiguousarray(mem3, np.float32))


# revision 3
# speedup vs baseline: 1.2142x; 1.2142x over previous
"""Trainium2 Bass kernel for nn_AttentionSpikingNetwork (B=64, S=512).

Data-parallel over batch across 8 NeuronCores (8 batch elems per core).
Measured 373-445us HW exec (device-window dependent) vs the 540-553us
3-pass-fp32r baseline; rel err 1.372e-2 (budget 2e-2), zero spk3 flips,
bit-stable across runs.

Precision plan (validated against an exact numpy FP22/fp8 simulation of
the kernel numerics; the sim matched hardware to 7 digits on the all-fp32r
config, and each pass-drop below was sim-verified to keep zero spk3 flips;
the output norm is tiny - spk3 fires at 0.1% - so a single spk3 flip costs
1.9e-2 and the flip budget is effectively zero):
  - embed: hi pass in fp32r (wEh_m11 @ x_m11, contract zero-padded
    784->896 so all chunks run the fast 128-contraction) + BOTH lo
    compensation terms (wEh@xl + wEl@xh) in one fp8e4m3 DoubleRow psum
    group (halves the lo instruction count), sharing a 2^13 operand
    scale; combined at the drain as spike = hi > (0.5-bE) - 2^-13*lo.
    DoubleRow operands are host-packed [128, 2, *] tiles (contract
    784->1024).
  - V, attention, cur3: single fp32r hi pass (sim: 6.4e-3 combined).
  - cur2: full 3-pass fp32r (dropping costs 1.6e-2; an fp8-DR variant of
    its lo passes measured slower end-to-end and noisier - not used).
  - Q/K: single-pass packed weight (Wq|Wk in one 128-col block, one
    5-matmul chain; drains zero-pad rows 64:127 once and reuse); scores
    single-pass FP22 (softmax cancels the common-mode rounding).
Structure:
  - Activations flow transposed ([feat, seq]); scores transposed (K @ Q.T);
    softmax without max-subtraction; exp written as F32R by the ACT engine
    directly (no DVE cast).
  - softmax denominator folded into the attention matmul: vh carries 8
    zero columns + an all-ones column 608, so attention chunk 4 (emitted
    first) yields den at psum partition 96 (a DVE-alignable base);
    reciprocal_approx_fast (den is a benign positive O(500) value) +
    gpsimd broadcast run under attention chunks 0-3.  The custom-DVE
    reciprocal ignores psum partition offsets, so den is first copied to
    a base-0 SBUF tile.
  - bv folded into b2 host-side (b2' = b2 + W2 @ bv) - no attention bias
    stt at all; the skip-add is a plain tensor_tensor.
  - V-psum and s2h drains run on the ACT engine (Copy) to relieve DVE.
  - elem b+1's embed j-chunks are emitted between b's scores and attention
    as PE filler for the exp/normalize chains; embed is j-major so only
    4 psum banks hold it (2 hi double-buffered + 2 lo), leaving 5 banks
    for the qk/V/scores/attn/cur2/cur3 rotation.
"""
import os
import sys

for _p in ("/opt/trn_rl_repo", "/root/.axon_site/_ro/trn_rl_repo"):
    if os.path.isdir(_p) and _p not in sys.path:
        sys.path.insert(0, _p)

import numpy as np
from contextlib import ExitStack

import concourse.bass as bass
import concourse.bass_isa as bass_isa
import concourse.bacc as bacc
import concourse.mybir as mybir
import concourse.tile as tile
from concourse.bass_utils import run_bass_kernel_spmd

F32 = mybir.dt.float32
F32R = mybir.dt.float32r
F8 = mybir.dt.float8e4
AF = mybir.ActivationFunctionType
OP = mybir.AluOpType
DR = mybir.MatmulPerfMode.DoubleRow

NCORES = 8
B, S, DIN, DEMB, DQK, DH2, DOUT = 64, 512, 784, 600, 64, 200, 10
NB = B // NCORES   # batch elems per core
DINP = 896         # DIN zero-padded so every hi chunk contracts 128 (the
                   # 16-wide tail chunk clocks at 460ns vs 277ns)
DIN8 = 1024        # DIN padded for fp8 DoubleRow chunks of 256
NKK = DIN8 // 256  # 4 DoubleRow contract chunks
DEBUG_S1 = False
DEBUG_MID = False
SC8 = np.float32(8192.0)   # 2^13 lo-operand scale
ISC8 = float(1.0 / 8192.0)


def _chunks(total, step=128):
    return [(i, min(step, total - i)) for i in range(0, total, step)]

CH_DIN = _chunks(DINP)   # 7 chunks of 128 (zero-padded from 784)
CH_EMB = _chunks(DEMB)   # 5
CH_H2 = _chunks(DH2)     # 2
CH_S = _chunks(S)        # 4
CH_VN = [(0, 344), (344, 256)]  # V free-dim split; both >=256 keeps fp32r full-rate


def _pad32(n):
    """DoubleRow LDWEIGHTS needs 32-aligned column counts (88 -> 96)."""
    return (n + 31) // 32 * 32


def round_m11(a):
    """Round fp32 to 11 explicit mantissa bits (fp32r/FP22 grid), RNE."""
    a = np.ascontiguousarray(a, np.float32)
    u = a.view(np.uint32).astype(np.uint64)
    r = (u + 0x7FF + ((u >> 12) & 1)) & np.uint64(0xFFFFF000)
    return r.astype(np.uint32).view(np.float32)


def _split(a):
    hi = round_m11(a)
    lo = (a.astype(np.float32) - hi).astype(np.float32)
    return hi, lo


def build_nc(nb=NB):
    nc = bacc.Bacc()

    def par(name, shape, dt=F32R, out=False):
        return nc.declare_dram_parameter(name, list(shape), dt, isOutput=out)

    xh = par("xh", [nb, DINP, S])
    x8h = par("x8h", [nb, NKK, 128, 2, S], F8)   # e4m3(xh), DR-packed
    x8l = par("x8l", [nb, NKK, 128, 2, S], F8)   # e4m3(xl*2^13), DR-packed
    wEh = par("wEh", [DINP * DEMB])
    demb8 = sum(_pad32(cn) for _, cn in CH_EMB)  # 608: col-padded chunks
    wE8h = par("wE8h", [NKK * 256 * demb8], F8)  # e4m3(wEh), DR blocks
    wE8l = par("wE8l", [NKK * 256 * demb8], F8)  # e4m3(wEl*2^13), DR blocks
    wQK = par("wQK", [DEMB, 128])
    wVh = par("wVh", [DEMB, DEMB])
    w2h = par("w2h", [DEMB * DH2])
    w2l = par("w2l", [DEMB * DH2])
    w3h = par("w3h", [DH2, DOUT])
    bE = par("bE", [DEMB, 1], F32)
    bQK = par("bQK", [128, 1], F32)
    b2 = par("b2", [DH2, 1], F32)
    b3 = par("b3", [DOUT, 1], F32)
    if DEBUG_S1:
        os1 = par("os1", [nb, DEMB, S], F32, out=True)
    if DEBUG_MID:
        oqh = par("oqh", [128, S], F32, out=True)
        okh = par("okh", [128, S], F32, out=True)
        opth = par("opth", [128, S], F32, out=True)
        oinv = par("oinv", [128, S], F32, out=True)
        os2h = par("os2h", [DEMB, S], F32, out=True)
        ospk2 = par("ospk2", [DH2, S], F32, out=True)
    os_ = par("os", [nb, DOUT, S], F32, out=True)
    om_ = par("om", [nb, DOUT, S], F32, out=True)

    with ExitStack() as ctx:
        tc = ctx.enter_context(tile.TileContext(nc))
        wp = ctx.enter_context(tc.tile_pool(name="wp", bufs=1))
        xp = ctx.enter_context(tc.tile_pool(name="xp", bufs=2))
        sp = ctx.enter_context(tc.tile_pool(name="sp", bufs=1))
        outp = ctx.enter_context(tc.tile_pool(name="outp", bufs=1))
        ps_em = ctx.enter_context(tc.tile_pool(name="ps_em", bufs=1, space="PSUM"))
        ps = ctx.enter_context(tc.tile_pool(name="ps", bufs=5, space="PSUM"))

        # ---- resident weights / consts ----
        # Weight DMAs are emitted j-major so batch elem 0's first embed
        # j-chunk has its blocks within ~1MB of DMA; the rest stream in
        # during elem 0's embed compute.
        wEh_t = {}     # (k, j) -> [kn, cn] f32r block
        wE8h_t = {}    # (kk, j) -> [128, 2, cn] fp8 DR block
        wE8l_t = {}

        def _emit_embed_weight_dmas():
            offs_h = {}
            off = 0
            for k, (k0, kn) in enumerate(CH_DIN):
                for j, (c0, cn) in enumerate(CH_EMB):
                    offs_h[(k, j)] = (off, kn, cn)
                    off += kn * cn
            offs_8 = {}
            off = 0
            for kk in range(NKK):
                for j, (c0, cn) in enumerate(CH_EMB):
                    offs_8[(kk, j)] = (off, _pad32(cn))
                    off += 256 * _pad32(cn)
            for j, (c0, cn) in enumerate(CH_EMB):
                for k, (k0, kn) in enumerate(CH_DIN):
                    t = wp.tile([kn, cn], F32R, name=f"wEh_{k}_{j}",
                                tag=f"wEh_{k}_{j}")
                    o, _, _ = offs_h[(k, j)]
                    nc.scalar.dma_start(out=t, in_=wEh[o:o + kn * cn].rearrange(
                        "(a b) -> a b", b=cn))
                    wEh_t[(k, j)] = t
                for kk in range(NKK):
                    for nm, dram, store in (("wE8h", wE8h, wE8h_t),
                                            ("wE8l", wE8l, wE8l_t)):
                        o, cp = offs_8[(kk, j)]
                        t = wp.tile([128, 2, cp], F8, name=f"{nm}_{kk}_{j}",
                                    tag=f"{nm}_{kk}_{j}")
                        nc.scalar.dma_start(
                            out=t, in_=dram[o:o + 256 * cp].rearrange(
                                "(p a b) -> p a b", a=2, b=cp))
                        store[(kk, j)] = t

        def wtiles(dram, chs, width, nm):
            hs = []
            for i, (c0, cn) in enumerate(chs):
                t = wp.tile([cn, width], F32R, name=f"{nm}{i}", tag=f"{nm}{i}")
                nc.scalar.dma_start(out=t, in_=dram[c0:c0 + cn, :])
                hs.append(t)
            return hs

        def wtiles2(dram, rchs, cchs, nm):
            out = {}
            off = 0
            for i, (r0, rn) in enumerate(rchs):
                for j, (c0, cn) in enumerate(cchs):
                    t = wp.tile([rn, cn], F32R, name=f"{nm}_{i}_{j}",
                                tag=f"{nm}_{i}_{j}")
                    nc.scalar.dma_start(
                        out=t, in_=dram[off:off + rn * cn].rearrange(
                            "(a b) -> a b", b=cn))
                    out[(i, j)] = t
                    off += rn * cn
            return out

        def btiles(dram, chs, nm):
            hs = []
            for i, (c0, cn) in enumerate(chs):
                t = wp.tile([cn, 1], F32, name=f"{nm}{i}", tag=f"{nm}{i}")
                nc.scalar.dma_start(out=t, in_=dram[c0:c0 + cn, :])
                hs.append(t)
            return hs

        _rest = {}

        def _load_rest():
            _rest["wQK"] = wtiles(wQK, CH_EMB, 128, "wQK")
            _rest["bQK"] = btiles(bQK, [(0, 128)], "bQK")[0]
            _rest["wVh"] = wtiles(wVh, CH_EMB, DEMB, "wVh")
            _rest["w2h"] = wtiles2(w2h, CH_EMB, CH_H2, "w2h")
            _rest["w2l"] = wtiles2(w2l, CH_EMB, CH_H2, "w2l")
            _rest["b2"] = btiles(b2, CH_H2, "b2")
            _rest["w3h"] = wtiles(w3h, CH_H2, DOUT, "w3h")
            _rest["b3"] = btiles(b3, [(0, DOUT)], "b3")[0]

        bE_t = btiles(bE, CH_EMB, "bE")

        MM = nc.tensor.matmul

        st = [dict() for _ in range(nb)]

        # ---- embed: j-major; per j-chunk one fp32r hi psum + one fp8-DR
        # lo psum, combined at the drain ----
        def emit_embed_x(b):
            xh_t = []
            for k, (k0, kn) in enumerate(CH_DIN):
                t = xp.tile([kn, S], F32R, name=f"xh{k}", tag=f"xh{k}")
                nc.sync.dma_start(out=t, in_=xh[b, k0:k0 + kn, :])
                xh_t.append(t)
            x8h_t, x8l_t = [], []
            for kk in range(NKK):
                th = xp.tile([128, 2, S], F8, name=f"x8h{kk}", tag=f"x8h{kk}")
                nc.sync.dma_start(out=th, in_=x8h[b, kk])
                x8h_t.append(th)
                tl = xp.tile([128, 2, S], F8, name=f"x8l{kk}", tag=f"x8l{kk}")
                nc.sync.dma_start(out=tl, in_=x8l[b, kk])
                x8l_t.append(tl)
            st[b]["x"] = (xh_t, x8h_t, x8l_t)
            st[b]["s1"] = [None] * len(CH_EMB)

        def emit_embed_j(b, j):
            xh_t, x8h_t, x8l_t = st[b]["x"]
            c0, cn = CH_EMB[j]
            hi_ps = ps_em.tile([cn, S], F32, name=f"emh{j % 2}",
                               tag=f"emh{j % 2}")
            lo_ps = ps_em.tile([_pad32(cn), S], F32, name="eml", tag="eml")
            nk = len(CH_DIN)
            for k in range(nk):
                MM(hi_ps, wEh_t[(k, j)], xh_t[k], start=(k == 0),
                   stop=(k == nk - 1))
            for kk in range(NKK):
                MM(lo_ps, wE8h_t[(kk, j)], x8l_t[kk], start=(kk == 0),
                   stop=False, perf_mode=DR)
                MM(lo_ps, wE8l_t[(kk, j)], x8h_t[kk], start=False,
                   stop=(kk == NKK - 1), perf_mode=DR)
            # s1 = (hi + 2^-13*lo + bE > 0.5), done as hi > thr with
            # thr = (0.5 - bE) - 2^-13*lo so each DVE op reads one psum
            # (bE2 = 0.5 - bE is precomputed host-side)
            thr = sp.tile([cn, S], F32, name="emthr", tag="emthr", bufs=2)
            nc.vector.tensor_scalar(thr, lo_ps[0:cn, :], -ISC8, bE_t[j],
                                    OP.mult, OP.add)
            t = sp.tile([cn, S], F32R, name=f"s1_{j}", tag=f"s1_{j}", bufs=2)
            nc.vector.tensor_tensor(t, hi_ps, thr, OP.is_gt)
            if DEBUG_S1:
                nc.sync.dma_start(out=os1[b, c0:c0 + cn, :],
                                  in_=t.bitcast(F32))
            st[b]["s1"][j] = t

        def emit_qk(b):
            s1_t = st[b]["s1"]
            wQK_t = _rest["wQK"]
            q_ps = ps.tile([128, S], F32, name="qk_ps", tag="ps")
            n = len(CH_EMB)
            for i in range(n):
                MM(q_ps, wQK_t[i], s1_t[i], start=(i == 0), stop=(i == n - 1))
            # Q rows 0:64, K rows 64:128 -> zero-padded 128-contraction tiles
            qh = sp.tile([128, S], F32R, name="qh", tag="qh")
            kh = sp.tile([128, S], F32R, name="kh", tag="kh")
            if b == 0:
                # zero rows 64:128 once (reused across batch elems); memset
                # can't write F32R, so multiply finite psum rows by 0
                nc.vector.tensor_scalar(qh[64:128, :], q_ps[64:128, :],
                                        0.0, None, OP.mult)
                nc.vector.tensor_scalar(kh[64:128, :], q_ps[64:128, :],
                                        0.0, None, OP.mult)
            nc.vector.tensor_scalar(qh[0:64, :], q_ps[0:64, :],
                                    _rest["bQK"][0:64, :], None, OP.add)
            nc.vector.tensor_scalar(kh[0:64, :], q_ps[64:128, :],
                                    _rest["bQK"][64:128, :], None, OP.add)
            if DEBUG_MID and b == 0:
                nc.sync.dma_start(out=oqh[:, :], in_=qh.bitcast(F32))
                nc.sync.dma_start(out=okh[:, :], in_=kh.bitcast(F32))
            st[b].update(kh=kh, qh=qh)

        def emit_V(b):
            s1_t = st[b]["s1"]
            wVh_t = _rest["wVh"]
            vh_t = []
            for ti, (t0, tn) in enumerate(CH_S):
                v_ps = [ps.tile([tn, w], F32, name=f"v_ps{j}", tag="ps")
                        for j, (v0, w) in enumerate(CH_VN)]
                n = len(CH_EMB)
                for i in range(n):
                    lh = s1_t[i][:, t0:t0 + tn]
                    for j, (v0, w) in enumerate(CH_VN):
                        MM(v_ps[j], lh, wVh_t[i][:, v0:v0 + w],
                           start=(i == 0), stop=(i == n - 1))
                # 609-wide: cols 600:608 zero, col 608 ones — attention
                # chunk 4 then yields the softmax denominator at psum
                # partition 96 (a DVE-alignable base; 88 is not)
                vh = sp.tile([tn, DEMB + 9], F32R, name=f"vh{ti}",
                             tag=f"vh{ti}")
                for j, (v0, w) in enumerate(CH_VN):
                    nc.scalar.activation(vh[:, v0:v0 + w], v_ps[j], AF.Copy)
                nc.vector.tensor_scalar(vh[:, DEMB:DEMB + 8],
                                        v_ps[0][:, 0:8], 0.0, None, OP.mult)
                nc.vector.tensor_scalar(vh[:, DEMB + 8:DEMB + 9],
                                        v_ps[0][:, 0:1], 0.0, 1.0,
                                        OP.mult, OP.add)
                vh_t.append(vh)
            st[b]["vh"] = vh_t

        def emit_scores(b):
            qh, kh = st[b]["qh"], st[b]["kh"]
            pth_t = []
            for ti, (t0, tn) in enumerate(CH_S):
                scT_ps = ps.tile([tn, S], F32, name=f"scT_ps{ti}", tag="ps")
                MM(scT_ps, kh[:, t0:t0 + tn], qh, start=True, stop=True)
                expT = sp.tile([tn, S], F32R, name=f"expT{ti}",
                               tag=f"expT{ti}", bufs=2)
                nc.scalar.activation(expT, scT_ps, AF.Exp, scale=0.125)
                pth_t.append(expT)
            if DEBUG_MID and b == 0:
                nc.sync.dma_start(out=opth[:, :], in_=pth_t[0].bitcast(F32))
            st[b].update(pth=pth_t)

        def emit_attn_num(b):
            """attention numerator chunk 4 first (89 rows incl. den), then
            chunks 0-2; the den reciprocal+broadcast runs under them."""
            vh_t = st[b]["vh"]
            pth_t = st[b]["pth"]
            nt = len(CH_S)
            ao = {}
            for i in (4, 0, 1, 2):
                c0, cn = CH_EMB[i]
                w = cn + 9 if i == 4 else cn   # 88 V rows + 8 zeros + den
                t = ps.tile([w, S], F32, name=f"ao_ps{i}", tag="ps")
                for ti in range(nt):
                    MM(t, vh_t[ti][:, c0:c0 + w], pth_t[ti],
                       start=(ti == 0), stop=(ti == nt - 1))
                ao[i] = t
                if i == 4:
                    # copy den to a base-0 SBUF tile first: the custom-DVE
                    # reciprocal does not honor the psum partition offset
                    dent = sp.tile([1, S], F32, name="dent", tag="dent",
                                   bufs=2)
                    nc.vector.tensor_copy(dent, t[96:97, :])
                    invs = sp.tile([1, S], F32, name="invs", tag="invs",
                                   bufs=2)
                    nc.vector.reciprocal_approx_fast(out=invs, in_=dent)
                    invb = sp.tile([128, S], F32, name="invb", tag="invb",
                                   bufs=2)
                    nc.gpsimd.partition_broadcast(invb, invs)
                    if DEBUG_MID and b == 0:
                        nc.sync.dma_start(out=oinv[:, :], in_=invb)
                    st[b]["invb"] = invb
            st[b]["ao"] = ao

        def _drain_s2(b, i):
            s1_t = st[b]["s1"]
            invb = st[b]["invb"]
            ao = st[b]["ao"]
            c0, cn = CH_EMB[i]
            raw = sp.tile([cn, S], F32, name="s2raw", tag="s2raw", bufs=2)
            nc.vector.scalar_tensor_tensor(raw, ao[i][0:cn, :], 0.0,
                                           invb[0:cn, :], OP.add, OP.mult)
            # s2 = raw + s1 (bv folded into b2); f32 then split h/l
            f = sp.tile([cn, S], F32, name="s2f", tag="s2f", bufs=2)
            nc.vector.tensor_tensor(f, raw, s1_t[i].bitcast(F32), OP.add)
            h = sp.tile([cn, S], F32R, name=f"s2h{i}", tag=f"s2h{i}")
            nc.scalar.activation(h, f, AF.Copy)
            l = sp.tile([cn, S], F32R, name=f"s2l{i}", tag=f"s2l{i}")
            nc.vector.tensor_tensor(l, f, h.bitcast(F32), OP.subtract)
            if DEBUG_MID and b == 0:
                c0_, cn_ = CH_EMB[i]
                nc.sync.dma_start(out=os2h[c0_:c0_ + cn_, :],
                                  in_=h.bitcast(F32))
            st[b]["s2h"][i] = h
            st[b]["s2l"][i] = l

        def emit_attn_tail(b):
            vh_t = st[b]["vh"]
            pth_t = st[b]["pth"]
            nt = len(CH_S)
            ao = st[b]["ao"]
            st[b]["s2h"] = [None] * 5
            st[b]["s2l"] = [None] * 5

            # with 5 psum buffers ao[3] gets its own bank; drains follow
            c0, cn = CH_EMB[3]
            t = ps.tile([cn, S], F32, name="ao_ps3", tag="ps")
            for ti in range(nt):
                MM(t, vh_t[ti][:, c0:c0 + cn], pth_t[ti],
                   start=(ti == 0), stop=(ti == nt - 1))
            ao[3] = t
            for i in (4, 0, 1, 2, 3):
                _drain_s2(b, i)
            s2h_t, s2l_t = st[b]["s2h"], st[b]["s2l"]

            # cur2.T = W2 @ spk2_in.T (3-pass fp32r) -> spk2
            s2_t = []
            for hi, (h0, hn) in enumerate(CH_H2):
                c2_ps = ps.tile([hn, S], F32, name=f"c2_ps{hi}", tag="ps")
                n = len(CH_EMB)
                for i in range(n):
                    wh = _rest["w2h"][(i, hi)]
                    wl = _rest["w2l"][(i, hi)]
                    MM(c2_ps, wh, s2h_t[i], start=(i == 0), stop=False)
                    MM(c2_ps, wh, s2l_t[i], start=False, stop=False)
                    MM(c2_ps, wl, s2h_t[i], start=False, stop=(i == n - 1))
                t2 = sp.tile([hn, S], F32R, name=f"spk2_{hi}", tag=f"spk2_{hi}")
                nc.vector.tensor_scalar(t2, c2_ps, _rest["b2"][hi], 0.3,
                                        OP.add, OP.is_gt)
                if DEBUG_MID and b == 0:
                    nc.sync.dma_start(out=ospk2[h0:h0 + hn, :],
                                      in_=t2.bitcast(F32))
                s2_t.append(t2)

            # cur3.T = W3 @ spk2.T (single pass) -> outputs
            c3_ps = ps.tile([DOUT, S], F32, name="c3_ps", tag="ps")
            n = len(CH_H2)
            for hi in range(n):
                MM(c3_ps, _rest["w3h"][hi], s2_t[hi], start=(hi == 0),
                   stop=(hi == n - 1))
            spk3_t = outp.tile([DOUT, S], F32, name="spk3_t", tag="spk3_t")
            c3b_t = outp.tile([DOUT, S], F32, name="c3b_t", tag="c3b_t")
            mem3_t = outp.tile([DOUT, S], F32, name="mem3_t", tag="mem3_t")
            nc.vector.tensor_scalar(spk3_t, c3_ps, _rest["b3"], 0.3,
                                    OP.add, OP.is_gt)
            nc.vector.tensor_scalar(c3b_t, c3_ps, _rest["b3"], None, OP.add)
            nc.vector.scalar_tensor_tensor(mem3_t, spk3_t, -0.3, c3b_t,
                                           OP.mult, OP.add)
            nc.sync.dma_start(out=os_[b, :, :], in_=spk3_t)
            nc.sync.dma_start(out=om_[b, :, :], in_=mem3_t)

        _emit_embed_weight_dmas()
        emit_embed_x(0)
        for j in range(len(CH_EMB)):
            emit_embed_j(0, j)
            if j == 0:
                _load_rest()
        for b in range(nb):
            emit_qk(b)
            if b == nb - 1:
                emit_scores(b)
            emit_V(b)
            if b + 1 < nb:
                emit_embed_x(b + 1)
                emit_embed_j(b + 1, 0)
                emit_scores(b)
                emit_embed_j(b + 1, 1)
                emit_attn_num(b)
                emit_embed_j(b + 1, 2)
                emit_embed_j(b + 1, 3)
                emit_embed_j(b + 1, 4)
            else:
                emit_attn_num(b)
            emit_attn_tail(b)

    nc.finalize()
    return nc


_NC_CACHE = {}


def _get_nc(nb):
    if nb not in _NC_CACHE:
        _NC_CACHE[nb] = build_nc(nb)
    return _NC_CACHE[nb]


def _to_f8(a):
    import ml_dtypes
    return np.asarray(a, np.float32).astype(ml_dtypes.float8_e4m3)


def _pack_dr_x(xT):
    """[DIN, S] -> [NKK, 128, 2, S], contract rows p + 128*j + 256*kk."""
    out = np.zeros((NKK, 128, 2, S), np.float32)
    src = np.zeros((DIN8, S), np.float32)
    src[:DIN] = xT
    for kk in range(NKK):
        blk = src[kk * 256:(kk + 1) * 256]         # [256, S]
        out[kk] = blk.reshape(2, 128, S).transpose(1, 0, 2)
    return out


def _pack_dr_w(w):
    """[DIN8, DEMB] -> flat [(kk, j) blocks of [128, 2, pad32(cn)]]."""
    parts = []
    for kk in range(NKK):
        blk = w[kk * 256:(kk + 1) * 256]           # [256, DEMB]
        blk = blk.reshape(2, 128, -1).transpose(1, 0, 2)  # [128, 2, DEMB]
        for (c0, cn) in CH_EMB:
            cp = _pad32(cn)
            sub = np.zeros((128, 2, cp), np.float32)
            sub[:, :, :cn] = blk[:, :, c0:c0 + cn]
            parts.append(sub.ravel())
    return np.concatenate(parts)


def make_in_maps(x, We, be, Wq, bq, Wk, bk, Wv, bv, W2, b2, W3, b3,
                 ncores=NCORES):
    x = np.ascontiguousarray(x, np.float32)
    if x.max() > 1.0:
        x = (x * np.float32(1.0 / 255.0)).astype(np.float32)

    def _pack_blocks(w, rchs, cchs):
        return np.concatenate(
            [w[r0:r0 + rn, c0:c0 + cn].ravel()
             for (r0, rn) in rchs for (c0, cn) in cchs])

    wEh_f, wEl_f = _split(np.ascontiguousarray(We.T))     # [DIN, DEMB]
    wEh_p = np.zeros((DINP, DEMB), np.float32)
    wEh_p[:DIN] = wEh_f
    wQKh, _ = _split(np.concatenate(
        [np.ascontiguousarray(Wq.T), np.ascontiguousarray(Wk.T)], axis=1))
    wVh_f, _ = _split(np.ascontiguousarray(Wv.T))
    w2h_f, w2l_f = _split(np.ascontiguousarray(W2.T))
    w3h_f, _ = _split(np.ascontiguousarray(W3.T))

    wE_pad_h = np.zeros((DIN8, DEMB), np.float32)
    wE_pad_h[:DIN] = wEh_f
    wE_pad_l = np.zeros((DIN8, DEMB), np.float32)
    wE_pad_l[:DIN] = wEl_f * SC8

    # bv folded into b2: cur2 = W2 @ (attn+spk1+bv) + b2 = W2@(attn+spk1)+b2'
    b2_eff = (b2.astype(np.float64)
              + W2.astype(np.float64) @ bv.astype(np.float64))

    shared = dict(
        wEh=_pack_blocks(wEh_p, CH_DIN, CH_EMB),
        wE8h=_to_f8(_pack_dr_w(wE_pad_h)),
        wE8l=_to_f8(_pack_dr_w(wE_pad_l)),
        wQK=wQKh, wVh=wVh_f,
        w2h=_pack_blocks(w2h_f, CH_EMB, CH_H2),
        w2l=_pack_blocks(w2l_f, CH_EMB, CH_H2),
        w3h=w3h_f,
        bE=np.ascontiguousarray((0.5 - be).reshape(-1, 1), np.float32),
        bQK=np.ascontiguousarray(
            np.concatenate([bq, bk]).reshape(-1, 1), np.float32),
        b2=np.ascontiguousarray(b2_eff.reshape(-1, 1), np.float32),
        b3=np.ascontiguousarray(b3.reshape(-1, 1), np.float32),
    )

    nb = x.shape[0] // ncores
    in_maps = []
    for c in range(ncores):
        xs = x[c * nb:(c + 1) * nb]                       # [nb, S, DIN]
        xT = np.ascontiguousarray(xs.transpose(0, 2, 1))  # [nb, DIN, S]
        xh_, xl_ = _split(xT)
        xh_p = np.zeros((nb, DINP, S), np.float32)
        xh_p[:, :DIN] = xh_
        x8h_ = np.stack([_pack_dr_x(xh_[e]) for e in range(nb)])
        x8l_ = np.stack([_pack_dr_x(xl_[e] * SC8) for e in range(nb)])
        in_maps.append(dict(shared, xh=xh_p,
                            x8h=_to_f8(x8h_), x8l=_to_f8(x8l_)))
    return in_maps, nb


def kernel(x, We, be, Wq, bq, Wk, bk, Wv, bv, W2, b2, W3, b3, _trace=False):
    args = [np.asarray(a, np.float32) for a in
            (x, We, be, Wq, bq, Wk, bk, Wv, bv, W2, b2, W3, b3)]
    in_maps, nb = make_in_maps(*args)
    nc = _get_nc(nb)
    res = run_bass_kernel_spmd(nc, in_maps, list(range(NCORES)), trace=_trace)
    spk3 = np.concatenate([r["os"].transpose(0, 2, 1) for r in res.results], 0)
    mem3 = np.concatenate([r["om"].transpose(0, 2, 1) for r in res.results], 0)
    kernel.last_results = res
    return (np.ascontiguousarray(spk3, np.float32),
            np.ascontiguousarray(mem3, np.float32))
